# revision 15
# baseline (speedup 1.0000x reference)
"""Trainium2 Bass kernel for the pose-estimation loss (pm / t_center / t_depth).

Strategy
--------
pm[n] = mean_p | (pred_R[n]-gt_R[n]) @ obj_points[obj_id[n], p] |_1 / diam[obj_id[n]]

The data-dependent gather obj_points[obj_id] is folded into the matmul:
    Y[(i,n), p] = sum_{o,j} A[(o,j),(i,n)] * B[(o,j), p]
with A[(o,j),(i,n)] = [obj_id[n]==o] * dR[n,i,j]   (24 x 384, built on host)
     B[(o,j), p]    = obj_points[o, p, j]          (24 x 100000)

Points are split across the 8 cores (12500 each, padded to 12544).  The PSUM
abs+sum evacuation is the hard bottleneck on TRN2 (DVE and ACT each read
1 fp32/cycle/lane from PSUM; the verifier forbids dual-PSUM operands), so the
design maximizes evacuation efficiency:

  - 512-column matmuls fill whole PSUM banks -> fully contiguous drains.
  - DVE (tensor_reduce Abs) drains from banks 0-3, ACT (activation Abs +
    accum_out) from banks 4-7, both double buffered, running concurrently.
  - Tiles are assigned to engines by a greedy balance of measured per-tile
    costs (DVE ~1141 ns, ACT ~1330 ns per [128,2,512] tile).
  - Row-groups g0..g3 hold 7/7/6/4.5 chunks; 4 concurrent matmuls per fill
    burst via tile_position=(32g, 0).
  - Per-tile partial sums land in acc columns; the cross-column and
    cross-core summation happens on host (which already sums over cores).
  - t_center/t_depth: host presubtracts d = gt-pred; two tiny ACT Abs-accum
    ops at the end of the ACT queue (off the critical path).

Per core output: out[128, NCOLS+2] = [per-tile pm partials..., t_center, t_depth].
Host: pm = sum_over_cores(sum_cols) / 100000 / diam[obj_id].
"""

import os
import sys

import numpy as np

os.environ.setdefault("MYCRO_LOCAL_CACHE", "1")
if "/opt/trn_rl_repo" not in sys.path:
    sys.path.insert(0, "/opt/trn_rl_repo")

# ---- problem constants (hardcoded, must match the reference) ----
N_SAMPLES = 128
NUM_OBJECTS = 8
NUM_POINTS = 100000
N_CORES = 8

PTS_PER_CORE = NUM_POINTS // N_CORES  # 12500
CHUNK = 512
PTS_PAD = 12544                       # 24.5 chunks of 512
A_COLS = 3 * 128                      # 384
ICHUNKS = 3
# chunks per row-group (g3 has 4 full + 1 half chunk)
GCH_COLS = [7 * CHUNK, 7 * CHUNK, 6 * CHUNK, 4 * CHUNK + 256]
GCOLS = A_COLS + 7 * CHUNK

# per-i drain tiles: (bank0 spec, bank1 spec or None); spec = (group, col0, width)
def _tile_list():
    tiles = []
    for k in range(7):
        tiles.append(((0, k * CHUNK, CHUNK), (1, k * CHUNK, CHUNK)))
    for k in range(4):
        tiles.append(((2, k * CHUNK, CHUNK), (3, k * CHUNK, CHUNK)))
    tiles.append(((2, 4 * CHUNK, CHUNK), (2, 5 * CHUNK, CHUNK)))
    tiles.append(((3, 4 * CHUNK, 256), None))
    return tiles


TILES = _tile_list()
NTILES = len(TILES)                   # 13
NCOLS = ICHUNKS * NTILES              # 39 pm accumulator columns
OUT_COLS = NCOLS + 2                  # + t_center, t_depth

_CACHE = {}


def _build_module():
    if "nc" in _CACHE:
        return _CACHE["nc"]

    from contextlib import ExitStack

    import concourse.bass as bass  # noqa: F401  (import registers engines)
    import concourse.tile as tile
    from concourse import bacc, mybir

    f32 = mybir.dt.float32
    bf16 = mybir.dt.bfloat16

    nc = bacc.Bacc("TRN2", target_bir_lowering=False, debug=False)

    abmat = nc.dram_tensor("abmat", [128, GCOLS], bf16, kind="ExternalInput").ap()
    dmat = nc.dram_tensor("dmat", [128, 3], f32, kind="ExternalInput").ap()
    out = nc.dram_tensor("out", [128, OUT_COLS], f32, kind="ExternalOutput").ap()

    # greedy engine balance constants (measured per-tile cadence, ns)
    _DVE_COST = {1024: 1160.0, 512: 620.0, 256: 500.0}
    _ACT_COST = {1024: 1300.0, 512: 1050.0, 256: 950.0}

    def dve_cost(w):
        return _DVE_COST[w]

    def act_cost(w):
        return _ACT_COST[w]

    with ExitStack() as ctx:
        tc = ctx.enter_context(tile.TileContext(nc))
        const = ctx.enter_context(tc.tile_pool(name="const", bufs=1))
        ps_dve = ctx.enter_context(tc.tile_pool(name="psd", bufs=2, space="PSUM"))
        ps_act = ctx.enter_context(tc.tile_pool(name="psa", bufs=2, space="PSUM"))

        ab_sb = const.tile([128, GCOLS], bf16)
        d_sb = const.tile([128, 3], f32)
        acc = const.tile([128, OUT_COLS], f32)
        dummy = const.tile([128, 2, CHUNK], bf16)
        warm = const.tile([128, 1], f32)

        nc.vector.memset(warm, 0.0)

        # Input DMA on sync, column-split so the first chunk lands early.
        nc.gpsimd.dma_start(out=d_sb, in_=dmat)
        c1 = A_COLS + 1 * CHUNK
        c2 = A_COLS + 4 * CHUNK
        nc.sync.dma_start(out=ab_sb[:, 0:c1], in_=abmat[:, 0:c1])
        nc.sync.dma_start(out=ab_sb[:, c1:c2], in_=abmat[:, c1:c2])
        nc.sync.dma_start(out=ab_sb[:, c2:], in_=abmat[:, c2:])
        # Load the Abs ACT table set now so it overlaps the input DMAs.
        nc.scalar.activation(out=warm, in_=warm, func=mybir.ActivationFunctionType.Abs)

        def mm(dst, g, col0, w, i):
            nc.tensor.matmul(
                dst,
                lhsT=ab_sb[32 * g : 32 * g + 24, i * 128 : (i + 1) * 128],
                rhs=ab_sb[32 * g : 32 * g + 24, A_COLS + col0 : A_COLS + col0 + w],
                start=True,
                stop=True,
                tile_position=(32 * g, 0),
            )

        col = 0
        t_dve = 0.0
        t_act = 0.0
        for i in range(ICHUNKS):
            for spec0, spec1 in TILES:
                w = spec0[2]
                width = w * (2 if spec1 is not None else 1)
                use_dve = (t_dve + dve_cost(width)) <= (t_act + act_cost(width))
                pool = ps_dve if use_dve else ps_act
                pt = pool.tile([128, 2, CHUNK], f32)
                mm(pt[:, 0, 0 : spec0[2]], spec0[0], spec0[1], spec0[2], i)
                if spec1 is not None:
                    mm(pt[:, 1, 0 : spec1[2]], spec1[0], spec1[1], spec1[2], i)
                    red_in = pt
                else:
                    red_in = pt[:, 0:1, 0:w]
                if use_dve:
                    t_dve += dve_cost(width)
                    nc.vector.tensor_reduce(
                        out=acc[:, col : col + 1], in_=red_in,
                        axis=mybir.AxisListType.XY, op=mybir.AluOpType.add,
                        apply_absolute_value=True,
                    )
                else:
                    t_act += act_cost(width)
                    # in-place PSUM out: ScalarE sits closer to PSUM, ~30ns
                    # cheaper per op than an SBUF dummy write, and the tile is
                    # dead after this instruction anyway.
                    nc.scalar.activation(
                        out=red_in,
                        in_=red_in,
                        func=mybir.ActivationFunctionType.Abs,
                        accum_out=acc[:, col : col + 1],
                    )
                col += 1

        assert col == NCOLS

        # t_site losses at the end of the ACT queue (off the critical path):
        # host provides d = gt - pred; t_center = |d0|+|d1|, t_depth = |d2|.
        nc.scalar.activation(
            out=dummy[:, 0, 0:2], in_=d_sb[:, 0:2],
            func=mybir.ActivationFunctionType.Abs,
            accum_out=acc[:, NCOLS : NCOLS + 1],
        )
        nc.scalar.activation(
            out=dummy[:, 0, 2:3], in_=d_sb[:, 2:3],
            func=mybir.ActivationFunctionType.Abs,
            accum_out=acc[:, NCOLS + 1 : NCOLS + 2],
        )

        nc.sync.dma_start(out=out, in_=acc)

    nc.compile()
    _CACHE["nc"] = nc
    return nc


def _prepare_in_maps(obj_id, gt_cam_R_m2c, pred_cam_R_m2c, gt_cam_t_m2c_site,
                     pred_cam_t_m2c_site, obj_points, obj_diameters):
    obj_id = np.asarray(obj_id).astype(np.int64)
    dR = (np.asarray(pred_cam_R_m2c, np.float32)
          - np.asarray(gt_cam_R_m2c, np.float32))          # [N, 3, 3] (i, j)
    pts = np.asarray(obj_points, np.float32)               # [8, P, 3]

    import ml_dtypes

    # A[(o,j), (i,n)] = [obj_id[n]==o] * dR[n, i, j]
    afull = np.zeros((NUM_OBJECTS, 3, 3, N_SAMPLES), np.float32)  # [o, j, i, n]
    afull[obj_id, :, :, np.arange(N_SAMPLES)] = dR.transpose(0, 2, 1)  # [n, j, i]
    a24 = afull.reshape(NUM_OBJECTS * 3, 3 * N_SAMPLES)    # rows (o,j), cols i*128+n

    # B rows (o,j), cols p
    b24 = pts.transpose(0, 2, 1).reshape(NUM_OBJECTS * 3, NUM_POINTS)

    d_host = (np.asarray(gt_cam_t_m2c_site, np.float32)
              - np.asarray(pred_cam_t_m2c_site, np.float32))  # [128, 3]

    gbounds = np.cumsum([0] + GCH_COLS)
    in_maps = []
    for c in range(N_CORES):
        bc = np.zeros((24, PTS_PAD), np.float32)
        bc[:, :PTS_PER_CORE] = b24[:, c * PTS_PER_CORE : (c + 1) * PTS_PER_CORE]
        ab = np.zeros((128, GCOLS), np.float32)
        for g in range(4):
            r = 32 * g
            ab[r : r + 24, 0:A_COLS] = a24
            w = gbounds[g + 1] - gbounds[g]
            ab[r : r + 24, A_COLS : A_COLS + w] = bc[:, gbounds[g] : gbounds[g + 1]]
        in_maps.append({
            "abmat": np.ascontiguousarray(ab).astype(ml_dtypes.bfloat16),
            "dmat": d_host,
        })
    return in_maps, obj_id, np.asarray(obj_diameters, np.float32)


def _postprocess(results, obj_id, obj_diameters):
    pm_sum = np.zeros(N_SAMPLES, np.float64)
    for c in range(N_CORES):
        pm_sum += results[c]["out"][:, 0:NCOLS].astype(np.float64).sum(axis=1)
    pm = (pm_sum / NUM_POINTS / obj_diameters[obj_id].astype(np.float64)).astype(
        np.float32)
    t_center = results[0]["out"][:, NCOLS].astype(np.float32)
    t_depth = results[0]["out"][:, NCOLS + 1].astype(np.float32)
    return pm, t_center, t_depth


def run(inputs, trace=False):
    """Run on the 8 NeuronCores. Returns ((pm, t_center, t_depth), BassKernelResults)."""
    from concourse.bass_utils import run_bass_kernel_spmd

    nc = _build_module()
    in_maps, obj_id, diam = _prepare_in_maps(**inputs)
    res = run_bass_kernel_spmd(nc, in_maps, list(range(N_CORES)), trace=trace)
    return _postprocess(res.results, obj_id, diam), res


def run_sim(inputs):
    """CoreSim path (numerics check without hardware)."""
    from concourse.bass_interp import CoreSim

    nc = _build_module()
    in_maps, obj_id, diam = _prepare_in_maps(**inputs)
    results = []
    for c in range(N_CORES):
        sim = CoreSim(nc)
        for name, val in in_maps[c].items():
            sim.tensor(name)[:] = val
        sim.simulate(check_with_hw=False)
        results.append({"out": np.array(sim.tensor("out"))})
    return _postprocess(results, obj_id, diam)


def kernel(**inputs):
    (pm, t_center, t_depth), _ = run(inputs, trace=False)
    return pm, t_center, t_depth
